# revision 76
# baseline (speedup 1.0000x reference)
"""DenseCapsLayer Trainium2 kernel.

Math (per (n, a) pair; A=32 input capsule types, B=32 output, P=4, hw=256):
  votes v[h,b] = W[a,b] @ M[h]  (4x4 matmuls) -- NEVER materialized (256MB).
  Routing reduces to small per-pair contractions:
    Mbar[b]   = sum_h c[h,b] * M[h]          (c = softmax over h of L)
    S[b]      = W[a,b] @ Mbar[b]
    n2[b]     = |S[b]|^2 = <Mbar[b], G[a,b] @ Mbar[b]>,  G = W^T W  (host-precomputed)
    Pout[b]   = f(n2) * S[b]                  (squash factor f)
    U[b]      = W^T Pout[b] = f * G @ Mbar[b]
    L        += M @ U^T  (so L_t = M @ Ubar_t^T with Ubar = cumulative sum of U)
  Final output = Pout at iter 2.

Sharding: data-parallel over batch: core c handles n in {2c, 2c+1} (NL=2), all
32 a's. Per-core layout: 16 "groups" g = j*2 + nl (j = a-block of 4, nl =
local n); partitions = (aL, b) = aL*32 + b with aL = a - 4j.
"""

import numpy as np
import ml_dtypes

import concourse.bass as bass
import concourse.bacc as bacc
import concourse.mybir as mybir
import concourse.tile as tile
from concourse.bass_utils import run_bass_kernel_spmd

F32 = mybir.dt.float32
F16 = mybir.dt.float16
BF16 = mybir.dt.bfloat16

A, B, P, ITERS = 32, 32, 4, 3
PS = P * P                      # 16
BATCH, OH, OW = 16, 16, 16
HW = OH * OW                    # 256
NCORES = 8
NL = BATCH // NCORES            # 2 local batch items per core
J = A // 4                      # 8 groups of 4 a's
G = J * NL                      # 16 (g = j*NL + nl)
NB = 4                          # g-batches for L/exp processing (4 g each)
EPS = 1e-8

AF = mybir.ActivationFunctionType
ALU = mybir.AluOpType
AX = mybir.AxisListType


# ---------------------------------------------------------------- device code
import os as _os
_STOP = _os.environ.get("K_STOP", "")
# warm-trickle sizes (dummy PE matmuls filling engine gaps); see warm()
_WN = [int(v) for v in _os.environ.get(
    "K_WARM", "60,28,20,4,0,0").split(",")]


def _emit(tc, xs16t, xall, wga, wws, o32):
    nc = tc.nc

    dbg_view = o32.rearrange("n a b k -> (n a b k)") \
                  .rearrange("(p f) -> p f", f=256)

    def dump(src):
        # debug: copy a (128, 256) fp32 AP to the output
        nc.sync.dma_start(out=dbg_view, in_=src)

    with (
        tc.tile_pool(name="inp", bufs=1) as inp,
        tc.tile_pool(name="state", bufs=1) as state,
        tc.tile_pool(name="work", bufs=3) as work,
        tc.tile_pool(name="small", bufs=2) as small,
        tc.tile_pool(name="lps", bufs=2, space="PSUM") as lps_pool,
        tc.tile_pool(name="mbps", bufs=1, space="PSUM") as mbps_pool,
        tc.tile_pool(name="utps", bufs=1, space="PSUM") as utps_pool,
    ):
        # ---------------- persistent inputs in SBUF (batched DMAs)
        # X blob: [128, (hl, nl, ch, a*kq)] bf16, four slab DMAs.  The hi
        # slabs load first (t=0 Mb uses hi only); lo slabs defer behind the
        # weight/MT loads since they are first needed at t=1.
        # t=0 coupling is uniform, so Mbar_0 = mean_h x is a host-side
        # input statistic (exact f32) -- loads in one tiny DMA and unblocks
        # the whole t=0 chain from the X load.
        # GA+MB0 ride in one f16 blob, first in the queue: the t=0 chain
        # needs exactly these two and nothing else
        GM = inp.tile([128, G * 64 + G * PS], F16, tag="gm")
        nc.sync.dma_start(out=GM[:], in_=wga[:, :])
        GA = GM[:, 0:G * 64]
        MB0 = GM[:, G * 64:G * 64 + G * PS]
        X = inp.tile([128, 2 * 2 * NL * A * PS], BF16, tag="xall")
        for nl in range(NL):
            nc.sync.dma_start(out=X[:, nl * 1024:(nl + 1) * 1024],
                              in_=xall[nl])
        Xh = {}
        Xl = {}
        for ch in range(2):
            for nl in range(NL):
                Xh[nl, ch] = X[:, nl * 1024 + ch * 512:
                               nl * 1024 + ch * 512 + 512]
                Xl[nl, ch] = X[:, 2048 + nl * 1024 + ch * 512:
                               2048 + nl * 1024 + ch * 512 + 512]

        # MTall: (kq, g*1024 + aL*256 + h) fp16 -- M^T pre-transposed on the
        # HOST (static input), one DMA.  All matmul operands must live at
        # partition base 0 in this environment (mixing PE row-groups faults).
        MTall = inp.tile([PS, G * 4 * HW], F16, tag="mtall")
        nc.sync.dma_start(
            out=MTall[:].rearrange("p (g c) -> p g c", g=G),
            in_=xs16t.rearrange("g p c -> p g c"))
        MT16 = {g: MTall[:, g * 4 * HW:(g + 1) * 4 * HW] for g in range(G)}

        WS = inp.tile([128, G * 64], F16, tag="ws")
        nc.scalar.dma_start(out=WS[:], in_=wws[:, :])
        for nl in range(NL):
            nc.sync.dma_start(
                out=X[:, 2048 + nl * 1024:2048 + (nl + 1) * 1024],
                in_=xall[NL + nl])

        ones_bf = inp.tile([128, 128], BF16, tag="ones_bf")
        nc.gpsimd.memset(ones_bf[:], 1.0)
        ident = inp.tile([128, 128], F16, tag="ident")
        from concourse.masks import make_identity
        make_identity(nc, ident[:])
        onecol = inp.tile([128, 1], BF16, tag="onecol")
        nc.gpsimd.memset(onecol[:], 1.0)
        epsc = inp.tile([128, 1], F32, tag="epsc")
        nc.gpsimd.memset(epsc[:], EPS)

        # Preload the combined exp+ln activation table set once; otherwise the
        # table-load pass alternates exp_and_others / natural_log every iter
        # (~1.3us per reload).
        from concourse.hw_specs import get_activation_tables
        _tables = list(get_activation_tables(nc.m.arch).items())
        _set_id = next(i for i, (nm, fns) in enumerate(_tables)
                       if AF.Exp in fns and AF.Ln in fns)
        nc.scalar.add_instruction(mybir.InstLoadActFuncSet(
            name=nc.get_next_instruction_name(),
            ins=[], outs=[], act_func_set_id=_set_id))

        if _STOP == "setup":
            dump(WS[:, 0:256])
            return

        ubar_prev = None
        lps_tiles = {}
        el_tiles = {}

        def warm(n, out_ps):
            # dummy matmuls to keep the PE p-state ramped through engine gaps
            # (cold-start matmuls cost up to ~4x); results are overwritten.
            for _ in range(n):
                nc.tensor.matmul(out_ps[0:128, 0:256], ident[:],
                                 GA[:, 0:256], start=True, stop=True,
                                 skip_group_check=True)

        for t in range(ITERS):
            # -------- Mb matmuls (+ exp for t>0), processed in 4-g batches
            mb_ps0 = mbps_pool.tile([128, 8 * 64], F32, tag="mb0")
            mb_ps = [mb_ps0]
            den_ps = None
            if t > 0:
                mb_ps1 = mbps_pool.tile([128, 8 * 64], F32, tag="mb1")
                mb_ps.append(mb_ps1)
                den_ps = mbps_pool.tile([128, 16], F32, tag="den")
            if t == 0:
                # PE warmup: dummy matmuls keep the tensor engine busy
                # through the input-DMA window so the real Mb matmuls run at
                # ramped p-state (cold 512-col matmuls cost ~4x).
                for _w in range(_WN[0]):
                    nc.tensor.matmul(mb_ps[0][0:64, 0:64], ones_bf[:, 0:64],
                                     ones_bf[:, 0:64], start=True, stop=True,
                                     skip_group_check=True)
                # uniform coupling: Mb[nl] = sum_h X -- same values for every
                # j-block, so one 512-col ones-matmul per (nl, ch, h/l)
                pass  # Mbar_0 comes in via the MB0 input tile
            else:
                for bi in range(NB):
                    el = el_tiles[bi]  # exp'd at end of previous iteration
                    for gi in range(4):
                        g = bi * 4 + gi
                        nl, j = g // J, g % J
                        out_g = mb_ps[g // 8][:, (g % 8) * 64:
                                              (g % 8) * 64 + 64]
                        for ch in range(2):
                            lhsT = el[:, gi * 256 + ch * 128:
                                      gi * 256 + (ch + 1) * 128]
                            # denominator: accumulate ch0+ch1 in psum
                            dcol = (g // 8) * 8 + (g % 8)
                            nc.tensor.matmul(
                                den_ps[:, dcol:dcol + 1],
                                lhsT, onecol[:], start=(ch == 0),
                                stop=(ch == 1))
                            rx = Xh[nl, ch][:].rearrange(
                                "p (a kq) -> p a kq",
                                kq=PS)[:, 4 * j:4 * j + 4, :]
                            nc.tensor.matmul(out_g, lhsT, rx,
                                             start=(ch == 0), stop=False)
                            rxl = Xl[nl, ch][:].rearrange(
                                "p (a kq) -> p a kq",
                                kq=PS)[:, 4 * j:4 * j + 4, :]
                            nc.tensor.matmul(out_g, lhsT, rxl,
                                             start=False, stop=(ch == 1))

            # ================ post-Mb phase, pipelined per half H
            # (half H = g in [H*8, H*8+8) = local batch item nl == H, cols
            # [H*128, (H+1)*128) of all (g,kq)-shaped tensors)
            ub_halves = {}
            uta_halves = {}
            lp_iter = {}

            def get_lp(bi):
                if bi not in lp_iter:
                    lp_iter[bi] = lps_pool.tile([128, 1024], F32, tag="lps",
                                                name=f"lp{bi}")
                return lp_iter[bi]

            if t < 2:
                warm(_WN[1] if t == 0 else _WN[2], get_lp(0))

            for H in range(2):
                gsl = slice(0, 8)
                eng = nc.vector
                mbv = (mb_ps[H][:].rearrange("p (g c) -> p g c", c=64)
                       if t > 0 else None)
                if t < 2:
                    if t == 0:
                        mbar = MB0[:, H * 128:(H + 1) * 128]
                    else:
                        mbar = state.tile([128, 8 * PS], F16,
                                          tag=f"mbar{t}{H}",
                                          name=f"mbar{t}{H}")[:]
                    z = state.tile([128, 8 * PS], F16, tag=f"z{t}{H}")
                    ub = state.tile([128, 8 * PS], F16, tag=f"ubar{t}{H}")
                    uta = work.tile([PS, 8 * 128], F16, tag=f"uta{H}")
                else:
                    mbar = state.tile([128, 8 * PS], F16, tag=f"mbar32{H}", name=f"mbar32{H}")[:]
                    s = state.tile([128, 8 * PS], F32, tag=f"s{H}")
                    outsb = state.tile([128, 8 * PS], F32, tag=f"outsb{H}")
                mview = mbar.rearrange("p (g kq) -> p g kq", kq=PS)

                # ---- denominators for this half (ch-summed in psum)
                recd = None
                if t > 0:
                    recd = small.tile([128, 8], F32, tag=f"recd{H}")
                    nc.vector.reciprocal(recd[:], den_ps[:, H * 8:H * 8 + 8])

                # ---- extract diagonal blocks + normalize.  t>0: raw f32
                # copies (Act/DVE split) then ONE normalize mult -- cheaper
                # on DVE than four strided tensor_tensor ops.
                if t == 0:
                    pass  # mbar = MB0 slice, already normalized on host
                else:
                    # separate dst tiles per engine: same-tile writes from
                    # different engines serialize in the dep tracker
                    rawA = work.tile([128, 8 * PS], F32, tag=f"rawA{H}")
                    rawD = work.tile([128, 8 * PS], F32, tag=f"rawD{H}")
                    rA = rawA[:].rearrange("p (g kq) -> p g kq", kq=PS)
                    rD = rawD[:].rearrange("p (g kq) -> p g kq", kq=PS)
                    for aL in range(4):
                        src_ = mbv[aL * 32:(aL + 1) * 32, gsl,
                                   aL * 16:aL * 16 + 16]
                        if aL < 2:
                            nc.scalar.activation(rA[aL * 32:(aL + 1) * 32],
                                                 src_, AF.Identity)
                        else:
                            nc.vector.tensor_copy(rD[aL * 32:(aL + 1) * 32],
                                                  src_)
                    rbA = recd[0:64].unsqueeze(2).broadcast_to((64, 8, PS))
                    nc.vector.tensor_tensor(mview[0:64], rA[0:64], rbA,
                                            op=ALU.mult)
                    rbD = recd[64:128].unsqueeze(2).broadcast_to((64, 8, PS))
                    nc.vector.tensor_tensor(mview[64:128], rD[64:128], rbD,
                                            op=ALU.mult)

                if t < 2:
                    # ---- Z = G @ Mbar (fp16 elementwise + add tree)
                    tz = work.tile([128, 8 * 64], F16, tag=f"tz{H}")
                    tzv = tz[:].rearrange("p (g kp k q) -> p g kp k q",
                                          kp=4, k=4, q=4)
                    gav = GA.rearrange("p (g kp k q) -> p g kp k q",
                                          kp=4, k=4, q=4)[:, gsl]
                    min1 = mview.rearrange(
                        "p g (kp q) -> p g kp q", q=4) \
                        .unsqueeze(3).broadcast_to((128, 8, 4, 4, 4))
                    eng.tensor_tensor(tzv, gav, min1, op=ALU.mult)
                    tzs = tz[:].rearrange("p (g kp k q) -> p kp g k q",
                                          kp=4, k=4, q=4)
                    t01 = work.tile([128, 8 * PS], F16, tag=f"t01{H}")
                    t01v = t01[:].rearrange("p (g k q) -> p g k q", k=4, q=4)
                    eng.tensor_add(t01v, tzs[:, 0], tzs[:, 1])
                    t23 = work.tile([128, 8 * PS], F16, tag=f"t23{H}")
                    t23v = t23[:].rearrange("p (g k q) -> p g k q", k=4, q=4)
                    eng.tensor_add(t23v, tzs[:, 2], tzs[:, 3])
                    eng.tensor_add(z[:], t01[:], t23[:])
                    # ---- n2 = <Mbar, Z>
                    mz = state.tile([128, 8 * PS], F32, tag=f"mz{H}")
                    eng.tensor_mul(mz[:], mbar, z[:])
                    n2 = small.tile([128, 8], F32, tag=f"n2{H}")
                    nc.vector.tensor_reduce(
                        out=n2[:],
                        in_=mz[:].rearrange("p (g kq) -> p g kq", kq=PS),
                        op=ALU.add, axis=AX.X)
                else:
                    # ---- final S = W @ Mbar (f16 elementwise + add tree)
                    ts = work.tile([128, 8 * 64], F16, tag=f"ts{H}")
                    tsv = ts[:].rearrange("p (g k pp q) -> p g k pp q",
                                          k=4, pp=4, q=4)
                    wsv = WS[:].rearrange("p (g k pp q) -> p g k pp q",
                                          k=4, pp=4, q=4)[:, gsl]
                    min2 = mview.rearrange(
                        "p g (k q) -> p g k q", q=4) \
                        .unsqueeze(3).broadcast_to((128, 8, 4, 4, 4))
                    eng.tensor_tensor(tsv, wsv, min2, op=ALU.mult)
                    tsk = ts[:].rearrange("p (g k c) -> p k g c", k=4, c=16)
                    s01 = work.tile([128, 8 * PS], F16, tag=f"s01{H}")
                    eng.tensor_add(
                        s01[:].rearrange("p (g c) -> p g c", c=PS),
                        tsk[:, 0], tsk[:, 1])
                    s23 = work.tile([128, 8 * PS], F16, tag=f"s23{H}")
                    eng.tensor_add(
                        s23[:].rearrange("p (g c) -> p g c", c=PS),
                        tsk[:, 2], tsk[:, 3])
                    eng.tensor_add(s[:], s01[:], s23[:])
                    # n2 = |S|^2: square on Act (idle at t=2), reduce on DVE
                    mz = state.tile([128, 8 * PS], F32, tag=f"mz{H}")
                    nc.scalar.activation(mz[:], s[:], AF.Square)
                    n2 = small.tile([128, 8], F32, tag=f"n2{H}")
                    nc.vector.tensor_reduce(
                        out=n2[:],
                        in_=mz[:].rearrange("p (g kq) -> p g kq", kq=PS),
                        op=ALU.add, axis=AX.X)

                # ---- squash factor f = n2/(1+n2)/sqrt(n2+eps)
                tln = small.tile([128, 8], F32, tag=f"tln{H}")
                nc.scalar.activation(tln[:], n2[:], AF.Ln, bias=epsc[:])
                rr = small.tile([128, 8], F32, tag=f"rr{H}")
                nc.scalar.activation(rr[:], tln[:], AF.Exp, scale=-0.5)
                dd = small.tile([128, 8], F32, tag=f"dd{H}")
                nc.vector.tensor_scalar_add(dd[:], n2[:], 1.0)
                rec = small.tile([128, 8], F32, tag=f"rec{H}")
                nc.vector.reciprocal(rec[:], dd[:])
                ff = small.tile([128, 8], F32, tag=f"ff{H}")
                nc.vector.tensor_mul(ff[:], n2[:], rec[:])
                ff2 = small.tile([128, 8], F32, tag=f"ff2{H}")
                nc.vector.tensor_mul(ff2[:], ff[:], rr[:])
                fbc = ff2[:].unsqueeze(2).broadcast_to((128, 8, PS))

                if t == 2:
                    # ---- output Pout = f * S; half H is local batch item H
                    eng.tensor_tensor(
                        outsb[:].rearrange("p (g kq) -> p g kq", kq=PS),
                        s[:].rearrange("p (g kq) -> p g kq", kq=PS),
                        fbc, op=ALU.mult)
                    src_o = outsb[:].rearrange("p (jj kq) -> p jj kq",
                                               kq=PS)
                    dst_o = o32[H].rearrange("(jj aL) b kq -> (aL b) jj kq",
                                             jj=J)
                    nc.sync.dma_start(out=dst_o, in_=src_o)
                    continue

                # ---- U = f*Z ; Ubar += U
                ubv = ub[:].rearrange("p (g kq) -> p g kq", kq=PS)
                zv = z[:].rearrange("p (g kq) -> p g kq", kq=PS)
                if t == 0:
                    eng.tensor_tensor(ubv, zv, fbc, op=ALU.mult)
                else:
                    u16 = state.tile([128, 8 * PS], F16, tag=f"u16{H}")
                    eng.tensor_tensor(
                        u16[:].rearrange("p (g kq) -> p g kq", kq=PS),
                        zv, fbc, op=ALU.mult)
                    eng.tensor_add(ub[:], ubar_prev[H][:],
                                   u16[:])

                ub_halves[H] = ub
                uta_halves[H] = uta

            if t == 2:
                continue
            # ---- pass 2: all transposes + UT copies first, so half 1's
            # uta is ready before the exp pipeline needs L(bi2)
            for H in range(2):
                ub = ub_halves[H]
                utps = utps_pool.tile([PS, 8 * 128], F16, tag="utps")
                for gl in range(8):
                    nc.tensor.transpose(
                        utps[:, gl * 128:(gl + 1) * 128],
                        ub[:, gl * PS:(gl + 1) * PS], ident[:])
                warm(_WN[3], get_lp(H))
                nc.vector.tensor_copy(uta_halves[H][:], utps[:])

            # ---- pass 3: L matmuls + exp per 4-g batch; at t=1 do H1
            # first so t=2's later half gets its exps/Mb data sooner
            for H in ((1, 0) if t == 1 else (0, 1)):
                uta = uta_halves[H]
                ut16 = {g: uta[:, (g - H * 8) * 128:(g - H * 8 + 1) * 128]
                        for g in range(H * 8, H * 8 + 8)}
                for bi in (H * 2, H * 2 + 1):
                    lp = get_lp(bi)
                    lps_tiles[bi] = lp
                    for gi in range(4):
                        g = bi * 4 + gi
                        for ch in range(2):
                            for aL in range(4):
                                lhsT = MT16[g][0:PS,
                                               aL * 256 + ch * 128:
                                               aL * 256 + (ch + 1) * 128]
                                rhs = ut16[g][0:PS, aL * 32:(aL + 1) * 32]
                                nc.tensor.matmul(
                                    lp[:, gi * 256 + ch * 128 + aL * 32:
                                       gi * 256 + ch * 128 + (aL + 1) * 32],
                                    lhsT, rhs, start=True, stop=True)
                    elb = state.tile([128, 1024], BF16, tag=f"el{bi}")
                    nc.scalar.activation(elb[:], lp[:], AF.Exp)
                    el_tiles[bi] = elb
                if H == 0:
                    warm(_WN[4], get_lp(2))
                else:
                    warm(_WN[5], mb_ps[0])
            if t < 2:
                ubar_prev = ub_halves
            if _STOP == f"t{t}l":
                dmp = state.tile([128, 256], F32, tag="dmp")
                nc.vector.tensor_copy(dmp[:], lps_tiles[0][:, 0:256])
                dump(dmp[:])
                return


def _build_kernel():
    nc = bacc.Bacc("TRN2", target_bir_lowering=False, debug=False,
                   num_devices=NCORES)
    xs16t = nc.dram_tensor("xs16t", [G, PS, 4 * HW], F16,
                           kind="ExternalInput").ap()
    xall = nc.dram_tensor("xall", [2 * NL, 128, 2 * A * PS], BF16,
                          kind="ExternalInput").ap()
    wga = nc.dram_tensor("wga", [128, G * 64 + G * PS], F16,
                     kind="ExternalInput").ap()
    wws = nc.dram_tensor("wws", [128, G * 64], F16, kind="ExternalInput").ap()
    o32 = nc.dram_tensor("o32", [NL, A, B, PS], F32,
                         kind="ExternalOutput").ap()

    with tile.TileContext(nc) as tc:
        _emit(tc, xs16t, xall, wga, wws, o32)

    nc.compile()
    return nc


# ---------------------------------------------------------------- host side
def _host_weights(weights):
    W = np.asarray(weights, np.float32)                # (A, B, P, P)
    Gm = np.einsum("abpk,abpl->abkl", W, W)            # (A, B, 4, 4): G[k, kp]
    Gsw = np.swapaxes(Gm, 2, 3)                        # Gsw[a,b,kp,k]=Gm[k,kp]
    Wsw = np.swapaxes(W, 2, 3)                         # Wsw[a,b,k,pp]=W[pp,k]

    wga = np.zeros((4, B, G, 4, 4, 4), np.float32)     # (aL,b,g,kp,k,q)
    wws = np.zeros((4, B, G, 4, 4, 4), np.float32)     # (aL,b,g,k,pp,q)
    for g in range(G):
        j = g % J                                      # g = nl*8 + j
        wga[:, :, g] = Gsw[4 * j:4 * j + 4, :, :, :, None]
        wws[:, :, g] = Wsw[4 * j:4 * j + 4, :, :, :, None]
    wga = wga.reshape(4 * B, G * 64)
    wws = wws.reshape(4 * B, G * 64)
    return wga.astype(np.float16), wws.astype(np.float16)


def _host_prep(x, weights):
    xr = np.asarray(x, np.float32).reshape(BATCH, HW, A, PS)
    wga, wws = _host_weights(weights)

    in_maps = []
    for c in range(NCORES):
        xc = xr[c * NL:(c + 1) * NL]                   # (NL, HW, A, PS)
        xh = xc.astype(ml_dtypes.bfloat16)
        xl = (xc - xh.astype(np.float32)).astype(ml_dtypes.bfloat16)
        # xall[(hl, nl), p, (ch, a*kq)]: h = ch*128 + p
        xh_r = xh.reshape(NL, 2, 128, A * PS)          # (nl, ch, p, c)
        xl_r = xl.reshape(NL, 2, 128, A * PS)
        xa = np.empty((2, NL, 128, 2, A * PS), ml_dtypes.bfloat16)
        xa[0] = xh_r.transpose(0, 2, 1, 3)
        xa[1] = xl_r.transpose(0, 2, 1, 3)
        # xmb0[(aL b), (nl j kq)] = mean_h x[nl, h, 4j+aL, kq] (exact f32)
        m0 = (xc.astype(np.float64).sum(axis=1) / HW).astype(np.float32)
        m0 = m0.reshape(NL, J, 4, PS).transpose(2, 0, 1, 3)  # aL,nl,j,kq
        xmb0 = np.broadcast_to(
            m0[:, None], (4, 32, NL, J, PS)).reshape(128, G * PS)
        # xs16t[g, kq, aL*256 + h] = x[nl, h, 4j+aL, kq];  g = nl*8 + j
        xj = xc.reshape(NL, HW, J, 4, PS)              # (nl,h,j,aL,kq)
        xs16t = xj.transpose(0, 2, 4, 3, 1).astype(np.float16)  # nl,j,kq,aL,h
        in_maps.append({
            "xs16t": np.ascontiguousarray(xs16t.reshape(G, PS, 4 * HW)),
            "xall": np.ascontiguousarray(xa.reshape(2 * NL, 128, 2 * A * PS)),
            "wga": np.ascontiguousarray(np.concatenate(
                [wga, xmb0.astype(np.float16)], axis=1)),
            "wws": wws,
        })
    return in_maps


_NC_CACHE = {}


def kernel(x, weights):
    if "nc" not in _NC_CACHE:
        _NC_CACHE["nc"] = _build_kernel()
    nc = _NC_CACHE["nc"]
    in_maps = _host_prep(x, weights)
    res = run_bass_kernel_spmd(nc, in_maps, list(range(NCORES)))
    out = np.concatenate([res.results[c]["o32"] for c in range(NCORES)],
                         axis=0)
    return out.astype(np.float32)



# revision 79
# speedup vs baseline: 1.0015x; 1.0015x over previous
"""DenseCapsLayer Trainium2 kernel.

Math (per (n, a) pair; A=32 input capsule types, B=32 output, P=4, hw=256):
  votes v[h,b] = W[a,b] @ M[h]  (4x4 matmuls) -- NEVER materialized (256MB).
  Routing reduces to small per-pair contractions:
    Mbar[b]   = sum_h c[h,b] * M[h]          (c = softmax over h of L)
    S[b]      = W[a,b] @ Mbar[b]
    n2[b]     = |S[b]|^2 = <Mbar[b], G[a,b] @ Mbar[b]>,  G = W^T W  (host-precomputed)
    Pout[b]   = f(n2) * S[b]                  (squash factor f)
    U[b]      = W^T Pout[b] = f * G @ Mbar[b]
    L        += M @ U^T  (so L_t = M @ Ubar_t^T with Ubar = cumulative sum of U)
  Final output = Pout at iter 2.

Sharding: data-parallel over batch: core c handles n in {2c, 2c+1} (NL=2), all
32 a's. Per-core layout: 16 "groups" g = j*2 + nl (j = a-block of 4, nl =
local n); partitions = (aL, b) = aL*32 + b with aL = a - 4j.
"""

import numpy as np
import ml_dtypes

import concourse.bass as bass
import concourse.bacc as bacc
import concourse.mybir as mybir
import concourse.tile as tile
from concourse.bass_utils import run_bass_kernel_spmd

F32 = mybir.dt.float32
F16 = mybir.dt.float16
BF16 = mybir.dt.bfloat16

A, B, P, ITERS = 32, 32, 4, 3
PS = P * P                      # 16
BATCH, OH, OW = 16, 16, 16
HW = OH * OW                    # 256
NCORES = 8
NL = BATCH // NCORES            # 2 local batch items per core
J = A // 4                      # 8 groups of 4 a's
G = J * NL                      # 16 (g = j*NL + nl)
NB = 4                          # g-batches for L/exp processing (4 g each)
EPS = 1e-8

AF = mybir.ActivationFunctionType
ALU = mybir.AluOpType
AX = mybir.AxisListType


# ---------------------------------------------------------------- device code
import os as _os
_STOP = _os.environ.get("K_STOP", "")
# warm-trickle sizes (dummy PE matmuls filling engine gaps); see warm()
_WN = [int(v) for v in _os.environ.get(
    "K_WARM", "60,28,20,4,0,0").split(",")]


def _emit(tc, xs16t, xall, wga, wws, o32):
    nc = tc.nc

    dbg_view = o32.rearrange("n a b k -> (n a b k)") \
                  .rearrange("(p f) -> p f", f=256)

    def dump(src):
        # debug: copy a (128, 256) fp32 AP to the output
        nc.sync.dma_start(out=dbg_view, in_=src)

    with (
        tc.tile_pool(name="inp", bufs=1) as inp,
        tc.tile_pool(name="state", bufs=1) as state,
        tc.tile_pool(name="work", bufs=3) as work,
        tc.tile_pool(name="small", bufs=2) as small,
        tc.tile_pool(name="lps", bufs=2, space="PSUM") as lps_pool,
        tc.tile_pool(name="mbps", bufs=1, space="PSUM") as mbps_pool,
        tc.tile_pool(name="utps", bufs=1, space="PSUM") as utps_pool,
    ):
        # ---------------- persistent inputs in SBUF (batched DMAs)
        # X blob: [128, (hl, nl, ch, a*kq)] bf16, four slab DMAs.  The hi
        # slabs load first (t=0 Mb uses hi only); lo slabs defer behind the
        # weight/MT loads since they are first needed at t=1.
        # t=0 coupling is uniform, so Mbar_0 = mean_h x is a host-side
        # input statistic (exact f32) -- loads in one tiny DMA and unblocks
        # the whole t=0 chain from the X load.
        # GA+MB0 ride in one f16 blob, first in the queue: the t=0 chain
        # needs exactly these two and nothing else
        GM = inp.tile([128, G * 64 + G * PS], F16, tag="gm")
        nc.sync.dma_start(out=GM[:], in_=wga[:, :])
        GA = GM[:, 0:G * 64]
        MB0 = GM[:, G * 64:G * 64 + G * PS]
        X = inp.tile([128, 2 * 2 * NL * A * PS], BF16, tag="xall")
        for nl in range(NL):
            nc.sync.dma_start(out=X[:, nl * 1024:(nl + 1) * 1024],
                              in_=xall[nl])
        Xh = {}
        Xl = {}
        for ch in range(2):
            for nl in range(NL):
                Xh[nl, ch] = X[:, nl * 1024 + ch * 512:
                               nl * 1024 + ch * 512 + 512]
                Xl[nl, ch] = X[:, 2048 + nl * 1024 + ch * 512:
                               2048 + nl * 1024 + ch * 512 + 512]

        # MTall: (kq, g*1024 + aL*256 + h) fp16 -- M^T pre-transposed on the
        # HOST (static input), one DMA.  All matmul operands must live at
        # partition base 0 in this environment (mixing PE row-groups faults).
        MTall = inp.tile([PS, G * 4 * HW], F16, tag="mtall")
        nc.sync.dma_start(
            out=MTall[:].rearrange("p (g c) -> p g c", g=G),
            in_=xs16t.rearrange("g p c -> p g c"))
        MT16 = {g: MTall[:, g * 4 * HW:(g + 1) * 4 * HW] for g in range(G)}

        WS = inp.tile([128, G * 64], F16, tag="ws")
        nc.scalar.dma_start(out=WS[:], in_=wws[:, :])
        for nl in range(NL):
            nc.sync.dma_start(
                out=X[:, 2048 + nl * 1024:2048 + (nl + 1) * 1024],
                in_=xall[NL + nl])

        ones_bf = inp.tile([128, 128], BF16, tag="ones_bf")
        nc.gpsimd.memset(ones_bf[:], 1.0)
        ident = inp.tile([128, 128], F16, tag="ident")
        from concourse.masks import make_identity
        make_identity(nc, ident[:])
        onecol = inp.tile([128, 1], BF16, tag="onecol")
        nc.gpsimd.memset(onecol[:], 1.0)
        epsc = inp.tile([128, 1], F32, tag="epsc")
        nc.gpsimd.memset(epsc[:], EPS)

        # Preload the combined exp+ln activation table set once; otherwise the
        # table-load pass alternates exp_and_others / natural_log every iter
        # (~1.3us per reload).
        from concourse.hw_specs import get_activation_tables
        _tables = list(get_activation_tables(nc.m.arch).items())
        _set_id = next(i for i, (nm, fns) in enumerate(_tables)
                       if AF.Exp in fns and AF.Ln in fns)
        nc.scalar.add_instruction(mybir.InstLoadActFuncSet(
            name=nc.get_next_instruction_name(),
            ins=[], outs=[], act_func_set_id=_set_id))

        if _STOP == "setup":
            dump(WS[:, 0:256])
            return

        ubar_prev = None
        lps_tiles = {}
        el_tiles = {}

        def warm(n, out_ps):
            # dummy matmuls to keep the PE p-state ramped through engine gaps
            # (cold-start matmuls cost up to ~4x); results are overwritten.
            for _ in range(n):
                nc.tensor.matmul(out_ps[0:128, 0:256], ident[:],
                                 GA[:, 0:256], start=True, stop=True,
                                 skip_group_check=True)

        for t in range(ITERS):
            # -------- Mb matmuls (+ exp for t>0), processed in 4-g batches
            mb_ps0 = mbps_pool.tile([128, 8 * 64], F32, tag="mb0")
            mb_ps = [mb_ps0]
            den_ps = None
            if t > 0:
                mb_ps1 = mbps_pool.tile([128, 8 * 64], F32, tag="mb1")
                mb_ps.append(mb_ps1)
                den_ps = mbps_pool.tile([128, 16], F32, tag="den")
            if t == 0:
                # PE warmup: dummy matmuls keep the tensor engine busy
                # through the input-DMA window so the real Mb matmuls run at
                # ramped p-state (cold 512-col matmuls cost ~4x).
                for _w in range(_WN[0]):
                    nc.tensor.matmul(mb_ps[0][0:64, 0:64], ones_bf[:, 0:64],
                                     ones_bf[:, 0:64], start=True, stop=True,
                                     skip_group_check=True)
                # uniform coupling: Mb[nl] = sum_h X -- same values for every
                # j-block, so one 512-col ones-matmul per (nl, ch, h/l)
                pass  # Mbar_0 comes in via the MB0 input tile
            else:
                for bi in range(NB):
                    el = el_tiles[bi]  # exp'd at end of previous iteration
                    for gi in range(4):
                        g = bi * 4 + gi
                        nl, j = g // J, g % J
                        out_g = mb_ps[g // 8][:, (g % 8) * 64:
                                              (g % 8) * 64 + 64]
                        for ch in range(2):
                            lhsT = el[:, gi * 256 + ch * 128:
                                      gi * 256 + (ch + 1) * 128]
                            # denominator: accumulate ch0+ch1 in psum
                            dcol = (g // 8) * 8 + (g % 8)
                            nc.tensor.matmul(
                                den_ps[:, dcol:dcol + 1],
                                lhsT, onecol[:], start=(ch == 0),
                                stop=(ch == 1))
                            rx = Xh[nl, ch][:].rearrange(
                                "p (a kq) -> p a kq",
                                kq=PS)[:, 4 * j:4 * j + 4, :]
                            nc.tensor.matmul(out_g, lhsT, rx,
                                             start=(ch == 0), stop=False)
                            rxl = Xl[nl, ch][:].rearrange(
                                "p (a kq) -> p a kq",
                                kq=PS)[:, 4 * j:4 * j + 4, :]
                            nc.tensor.matmul(out_g, lhsT, rxl,
                                             start=False, stop=(ch == 1))

            if t == 2:
                # ---- final iteration: phase-split so H1's extraction runs
                # on DVE while Act does H0's Square/Ln/Exp round-trips
                fb_in = {}
                s_t = {}
                for H in range(2):
                    mbv = mb_ps[H][:].rearrange("p (g c) -> p g c", c=64)
                    mbar = state.tile([128, 8 * PS], F16, tag=f"mbar32{H}",
                                      name=f"mbar32{H}")[:]
                    mview = mbar.rearrange("p (g kq) -> p g kq", kq=PS)
                    recd = small.tile([128, 8], F32, tag=f"recd{H}")
                    nc.vector.reciprocal(recd[:], den_ps[:, H * 8:H * 8 + 8])
                    raw = work.tile([128, 8 * PS], F32, tag=f"raw{H}")
                    rawv = raw[:].rearrange("p (g kq) -> p g kq", kq=PS)
                    for aL in range(4):
                        src_ = mbv[aL * 32:(aL + 1) * 32, :,
                                   aL * 16:aL * 16 + 16]
                        nc.vector.tensor_copy(rawv[aL * 32:(aL + 1) * 32],
                                              src_)
                    rb = recd[:].unsqueeze(2).broadcast_to((128, 8, PS))
                    nc.vector.tensor_tensor(mview, rawv, rb, op=ALU.mult)
                    # S = W @ Mbar (f16 elementwise + add tree)
                    s = state.tile([128, 8 * PS], F32, tag=f"s{H}")
                    ts = work.tile([128, 8 * 64], F16, tag=f"ts{H}")
                    tsv = ts[:].rearrange("p (g k pp q) -> p g k pp q",
                                          k=4, pp=4, q=4)
                    wsv = WS[:].rearrange("p (g k pp q) -> p g k pp q",
                                          k=4, pp=4, q=4)[:, 0:8]
                    min2 = mview.rearrange("p g (k q) -> p g k q", q=4) \
                        .unsqueeze(3).broadcast_to((128, 8, 4, 4, 4))
                    nc.vector.tensor_tensor(tsv, wsv, min2, op=ALU.mult)
                    tsk = ts[:].rearrange("p (g k c) -> p k g c", k=4, c=16)
                    s01 = work.tile([128, 8 * PS], F16, tag=f"s01{H}")
                    nc.vector.tensor_add(
                        s01[:].rearrange("p (g c) -> p g c", c=PS),
                        tsk[:, 0], tsk[:, 1])
                    s23 = work.tile([128, 8 * PS], F16, tag=f"s23{H}")
                    nc.vector.tensor_add(
                        s23[:].rearrange("p (g c) -> p g c", c=PS),
                        tsk[:, 2], tsk[:, 3])
                    nc.vector.tensor_add(s[:], s01[:], s23[:])
                    mz = state.tile([128, 8 * PS], F32, tag=f"mz{H}")
                    nc.scalar.activation(mz[:], s[:], AF.Square)
                    n2 = small.tile([128, 8], F32, tag=f"n2{H}")
                    nc.vector.tensor_reduce(
                        out=n2[:],
                        in_=mz[:].rearrange("p (g kq) -> p g kq", kq=PS),
                        op=ALU.add, axis=AX.X)
                    # launch the Act half of the squash now; it completes
                    # while DVE handles the other half's extraction
                    tln = small.tile([128, 8], F32, tag=f"tln{H}")
                    nc.scalar.activation(tln[:], n2[:], AF.Ln, bias=epsc[:])
                    rr = small.tile([128, 8], F32, tag=f"rr{H}")
                    nc.scalar.activation(rr[:], tln[:], AF.Exp, scale=-0.5)
                    fb_in[H] = (n2, rr)
                    s_t[H] = s
                for H in range(2):
                    n2, rr = fb_in[H]
                    dd = small.tile([128, 8], F32, tag=f"dd{H}")
                    nc.vector.tensor_scalar_add(dd[:], n2[:], 1.0)
                    rec = small.tile([128, 8], F32, tag=f"rec{H}")
                    nc.vector.reciprocal(rec[:], dd[:])
                    ff = small.tile([128, 8], F32, tag=f"ff{H}")
                    nc.vector.tensor_mul(ff[:], n2[:], rec[:])
                    ff2 = small.tile([128, 8], F32, tag=f"ff2{H}")
                    nc.vector.tensor_mul(ff2[:], ff[:], rr[:])
                    fbc = ff2[:].unsqueeze(2).broadcast_to((128, 8, PS))
                    outsb = state.tile([128, 8 * PS], F32, tag=f"outsb{H}")
                    nc.vector.tensor_tensor(
                        outsb[:].rearrange("p (g kq) -> p g kq", kq=PS),
                        s_t[H][:].rearrange("p (g kq) -> p g kq", kq=PS),
                        fbc, op=ALU.mult)
                    src_o = outsb[:].rearrange("p (jj kq) -> p jj kq", kq=PS)
                    dst_o = o32[H].rearrange("(jj aL) b kq -> (aL b) jj kq",
                                             jj=J)
                    nc.sync.dma_start(out=dst_o, in_=src_o)
                continue

            # ================ post-Mb phase, pipelined per half H
            # (half H = g in [H*8, H*8+8) = local batch item nl == H, cols
            # [H*128, (H+1)*128) of all (g,kq)-shaped tensors)
            ub_halves = {}
            uta_halves = {}
            lp_iter = {}

            def get_lp(bi):
                if bi not in lp_iter:
                    lp_iter[bi] = lps_pool.tile([128, 1024], F32, tag="lps",
                                                name=f"lp{bi}")
                return lp_iter[bi]

            if t < 2:
                warm(_WN[1] if t == 0 else _WN[2], get_lp(0))

            for H in range(2):
                gsl = slice(0, 8)
                eng = nc.vector
                mbv = (mb_ps[H][:].rearrange("p (g c) -> p g c", c=64)
                       if t > 0 else None)
                if t < 2:
                    if t == 0:
                        mbar = MB0[:, H * 128:(H + 1) * 128]
                    else:
                        mbar = state.tile([128, 8 * PS], F16,
                                          tag=f"mbar{t}{H}",
                                          name=f"mbar{t}{H}")[:]
                    z = state.tile([128, 8 * PS], F16, tag=f"z{t}{H}")
                    ub = state.tile([128, 8 * PS], F16, tag=f"ubar{t}{H}")
                    uta = work.tile([PS, 8 * 128], F16, tag=f"uta{H}")
                else:
                    mbar = state.tile([128, 8 * PS], F16, tag=f"mbar32{H}", name=f"mbar32{H}")[:]
                    s = state.tile([128, 8 * PS], F32, tag=f"s{H}")
                    outsb = state.tile([128, 8 * PS], F32, tag=f"outsb{H}")
                mview = mbar.rearrange("p (g kq) -> p g kq", kq=PS)

                # ---- denominators for this half (ch-summed in psum)
                recd = None
                if t > 0:
                    recd = small.tile([128, 8], F32, tag=f"recd{H}")
                    nc.vector.reciprocal(recd[:], den_ps[:, H * 8:H * 8 + 8])

                # ---- extract diagonal blocks + normalize.  t>0: raw f32
                # copies (Act/DVE split) then ONE normalize mult -- cheaper
                # on DVE than four strided tensor_tensor ops.
                if t == 0:
                    pass  # mbar = MB0 slice, already normalized on host
                else:
                    # separate dst tiles per engine: same-tile writes from
                    # different engines serialize in the dep tracker
                    rawA = work.tile([128, 8 * PS], F32, tag=f"rawA{H}")
                    rawD = work.tile([128, 8 * PS], F32, tag=f"rawD{H}")
                    rA = rawA[:].rearrange("p (g kq) -> p g kq", kq=PS)
                    rD = rawD[:].rearrange("p (g kq) -> p g kq", kq=PS)
                    for aL in range(4):
                        src_ = mbv[aL * 32:(aL + 1) * 32, gsl,
                                   aL * 16:aL * 16 + 16]
                        if aL < 2:
                            nc.scalar.activation(rA[aL * 32:(aL + 1) * 32],
                                                 src_, AF.Identity)
                        else:
                            nc.vector.tensor_copy(rD[aL * 32:(aL + 1) * 32],
                                                  src_)
                    rbA = recd[0:64].unsqueeze(2).broadcast_to((64, 8, PS))
                    nc.vector.tensor_tensor(mview[0:64], rA[0:64], rbA,
                                            op=ALU.mult)
                    rbD = recd[64:128].unsqueeze(2).broadcast_to((64, 8, PS))
                    nc.vector.tensor_tensor(mview[64:128], rD[64:128], rbD,
                                            op=ALU.mult)

                if t < 2:
                    # ---- Z = G @ Mbar (fp16 elementwise + add tree)
                    tz = work.tile([128, 8 * 64], F16, tag=f"tz{H}")
                    tzv = tz[:].rearrange("p (g kp k q) -> p g kp k q",
                                          kp=4, k=4, q=4)
                    gav = GA.rearrange("p (g kp k q) -> p g kp k q",
                                          kp=4, k=4, q=4)[:, gsl]
                    min1 = mview.rearrange(
                        "p g (kp q) -> p g kp q", q=4) \
                        .unsqueeze(3).broadcast_to((128, 8, 4, 4, 4))
                    eng.tensor_tensor(tzv, gav, min1, op=ALU.mult)
                    tzs = tz[:].rearrange("p (g kp k q) -> p kp g k q",
                                          kp=4, k=4, q=4)
                    t01 = work.tile([128, 8 * PS], F16, tag=f"t01{H}")
                    t01v = t01[:].rearrange("p (g k q) -> p g k q", k=4, q=4)
                    eng.tensor_add(t01v, tzs[:, 0], tzs[:, 1])
                    t23 = work.tile([128, 8 * PS], F16, tag=f"t23{H}")
                    t23v = t23[:].rearrange("p (g k q) -> p g k q", k=4, q=4)
                    eng.tensor_add(t23v, tzs[:, 2], tzs[:, 3])
                    eng.tensor_add(z[:], t01[:], t23[:])
                    # ---- n2 = <Mbar, Z>
                    mz = state.tile([128, 8 * PS], F32, tag=f"mz{H}")
                    eng.tensor_mul(mz[:], mbar, z[:])
                    n2 = small.tile([128, 8], F32, tag=f"n2{H}")
                    nc.vector.tensor_reduce(
                        out=n2[:],
                        in_=mz[:].rearrange("p (g kq) -> p g kq", kq=PS),
                        op=ALU.add, axis=AX.X)
                else:
                    # ---- final S = W @ Mbar (f16 elementwise + add tree)
                    ts = work.tile([128, 8 * 64], F16, tag=f"ts{H}")
                    tsv = ts[:].rearrange("p (g k pp q) -> p g k pp q",
                                          k=4, pp=4, q=4)
                    wsv = WS[:].rearrange("p (g k pp q) -> p g k pp q",
                                          k=4, pp=4, q=4)[:, gsl]
                    min2 = mview.rearrange(
                        "p g (k q) -> p g k q", q=4) \
                        .unsqueeze(3).broadcast_to((128, 8, 4, 4, 4))
                    eng.tensor_tensor(tsv, wsv, min2, op=ALU.mult)
                    tsk = ts[:].rearrange("p (g k c) -> p k g c", k=4, c=16)
                    s01 = work.tile([128, 8 * PS], F16, tag=f"s01{H}")
                    eng.tensor_add(
                        s01[:].rearrange("p (g c) -> p g c", c=PS),
                        tsk[:, 0], tsk[:, 1])
                    s23 = work.tile([128, 8 * PS], F16, tag=f"s23{H}")
                    eng.tensor_add(
                        s23[:].rearrange("p (g c) -> p g c", c=PS),
                        tsk[:, 2], tsk[:, 3])
                    eng.tensor_add(s[:], s01[:], s23[:])
                    # n2 = |S|^2: square on Act (idle at t=2), reduce on DVE
                    mz = state.tile([128, 8 * PS], F32, tag=f"mz{H}")
                    nc.scalar.activation(mz[:], s[:], AF.Square)
                    n2 = small.tile([128, 8], F32, tag=f"n2{H}")
                    nc.vector.tensor_reduce(
                        out=n2[:],
                        in_=mz[:].rearrange("p (g kq) -> p g kq", kq=PS),
                        op=ALU.add, axis=AX.X)

                # ---- squash factor f = n2/(1+n2)/sqrt(n2+eps)
                tln = small.tile([128, 8], F32, tag=f"tln{H}")
                nc.scalar.activation(tln[:], n2[:], AF.Ln, bias=epsc[:])
                rr = small.tile([128, 8], F32, tag=f"rr{H}")
                nc.scalar.activation(rr[:], tln[:], AF.Exp, scale=-0.5)
                dd = small.tile([128, 8], F32, tag=f"dd{H}")
                nc.vector.tensor_scalar_add(dd[:], n2[:], 1.0)
                rec = small.tile([128, 8], F32, tag=f"rec{H}")
                nc.vector.reciprocal(rec[:], dd[:])
                ff = small.tile([128, 8], F32, tag=f"ff{H}")
                nc.vector.tensor_mul(ff[:], n2[:], rec[:])
                ff2 = small.tile([128, 8], F32, tag=f"ff2{H}")
                nc.vector.tensor_mul(ff2[:], ff[:], rr[:])
                fbc = ff2[:].unsqueeze(2).broadcast_to((128, 8, PS))

                if t == 2:
                    # ---- output Pout = f * S; half H is local batch item H
                    eng.tensor_tensor(
                        outsb[:].rearrange("p (g kq) -> p g kq", kq=PS),
                        s[:].rearrange("p (g kq) -> p g kq", kq=PS),
                        fbc, op=ALU.mult)
                    src_o = outsb[:].rearrange("p (jj kq) -> p jj kq",
                                               kq=PS)
                    dst_o = o32[H].rearrange("(jj aL) b kq -> (aL b) jj kq",
                                             jj=J)
                    nc.sync.dma_start(out=dst_o, in_=src_o)
                    continue

                # ---- U = f*Z ; Ubar += U
                ubv = ub[:].rearrange("p (g kq) -> p g kq", kq=PS)
                zv = z[:].rearrange("p (g kq) -> p g kq", kq=PS)
                if t == 0:
                    eng.tensor_tensor(ubv, zv, fbc, op=ALU.mult)
                else:
                    u16 = state.tile([128, 8 * PS], F16, tag=f"u16{H}")
                    eng.tensor_tensor(
                        u16[:].rearrange("p (g kq) -> p g kq", kq=PS),
                        zv, fbc, op=ALU.mult)
                    eng.tensor_add(ub[:], ubar_prev[H][:],
                                   u16[:])

                ub_halves[H] = ub
                uta_halves[H] = uta

            if t == 2:
                continue
            # ---- pass 2: all transposes + UT copies first, so half 1's
            # uta is ready before the exp pipeline needs L(bi2)
            for H in range(2):
                ub = ub_halves[H]
                utps = utps_pool.tile([PS, 8 * 128], F16, tag="utps")
                for gl in range(8):
                    nc.tensor.transpose(
                        utps[:, gl * 128:(gl + 1) * 128],
                        ub[:, gl * PS:(gl + 1) * PS], ident[:])
                warm(_WN[3], get_lp(H))
                nc.vector.tensor_copy(uta_halves[H][:], utps[:])

            # ---- pass 3: L matmuls + exp per 4-g batch; at t=1 do H1
            # first so t=2's later half gets its exps/Mb data sooner
            for H in ((1, 0) if t == 1 else (0, 1)):
                uta = uta_halves[H]
                ut16 = {g: uta[:, (g - H * 8) * 128:(g - H * 8 + 1) * 128]
                        for g in range(H * 8, H * 8 + 8)}
                for bi in (H * 2, H * 2 + 1):
                    lp = get_lp(bi)
                    lps_tiles[bi] = lp
                    for gi in range(4):
                        g = bi * 4 + gi
                        for ch in range(2):
                            for aL in range(4):
                                lhsT = MT16[g][0:PS,
                                               aL * 256 + ch * 128:
                                               aL * 256 + (ch + 1) * 128]
                                rhs = ut16[g][0:PS, aL * 32:(aL + 1) * 32]
                                nc.tensor.matmul(
                                    lp[:, gi * 256 + ch * 128 + aL * 32:
                                       gi * 256 + ch * 128 + (aL + 1) * 32],
                                    lhsT, rhs, start=True, stop=True)
                    elb = state.tile([128, 1024], BF16, tag=f"el{bi}")
                    nc.scalar.activation(elb[:], lp[:], AF.Exp)
                    el_tiles[bi] = elb
                if H == 0:
                    warm(_WN[4], get_lp(2))
                else:
                    warm(_WN[5], mb_ps[0])
            if t < 2:
                ubar_prev = ub_halves
            if _STOP == f"t{t}l":
                dmp = state.tile([128, 256], F32, tag="dmp")
                nc.vector.tensor_copy(dmp[:], lps_tiles[0][:, 0:256])
                dump(dmp[:])
                return


def _build_kernel():
    nc = bacc.Bacc("TRN2", target_bir_lowering=False, debug=False,
                   num_devices=NCORES)
    xs16t = nc.dram_tensor("xs16t", [G, PS, 4 * HW], F16,
                           kind="ExternalInput").ap()
    xall = nc.dram_tensor("xall", [2 * NL, 128, 2 * A * PS], BF16,
                          kind="ExternalInput").ap()
    wga = nc.dram_tensor("wga", [128, G * 64 + G * PS], F16,
                     kind="ExternalInput").ap()
    wws = nc.dram_tensor("wws", [128, G * 64], F16, kind="ExternalInput").ap()
    o32 = nc.dram_tensor("o32", [NL, A, B, PS], F32,
                         kind="ExternalOutput").ap()

    with tile.TileContext(nc) as tc:
        _emit(tc, xs16t, xall, wga, wws, o32)

    nc.compile()
    return nc


# ---------------------------------------------------------------- host side
def _host_weights(weights):
    W = np.asarray(weights, np.float32)                # (A, B, P, P)
    Gm = np.einsum("abpk,abpl->abkl", W, W)            # (A, B, 4, 4): G[k, kp]
    Gsw = np.swapaxes(Gm, 2, 3)                        # Gsw[a,b,kp,k]=Gm[k,kp]
    Wsw = np.swapaxes(W, 2, 3)                         # Wsw[a,b,k,pp]=W[pp,k]

    wga = np.zeros((4, B, G, 4, 4, 4), np.float32)     # (aL,b,g,kp,k,q)
    wws = np.zeros((4, B, G, 4, 4, 4), np.float32)     # (aL,b,g,k,pp,q)
    for g in range(G):
        j = g % J                                      # g = nl*8 + j
        wga[:, :, g] = Gsw[4 * j:4 * j + 4, :, :, :, None]
        wws[:, :, g] = Wsw[4 * j:4 * j + 4, :, :, :, None]
    wga = wga.reshape(4 * B, G * 64)
    wws = wws.reshape(4 * B, G * 64)
    return wga.astype(np.float16), wws.astype(np.float16)


def _host_prep(x, weights):
    xr = np.asarray(x, np.float32).reshape(BATCH, HW, A, PS)
    wga, wws = _host_weights(weights)

    in_maps = []
    for c in range(NCORES):
        xc = xr[c * NL:(c + 1) * NL]                   # (NL, HW, A, PS)
        xh = xc.astype(ml_dtypes.bfloat16)
        xl = (xc - xh.astype(np.float32)).astype(ml_dtypes.bfloat16)
        # xall[(hl, nl), p, (ch, a*kq)]: h = ch*128 + p
        xh_r = xh.reshape(NL, 2, 128, A * PS)          # (nl, ch, p, c)
        xl_r = xl.reshape(NL, 2, 128, A * PS)
        xa = np.empty((2, NL, 128, 2, A * PS), ml_dtypes.bfloat16)
        xa[0] = xh_r.transpose(0, 2, 1, 3)
        xa[1] = xl_r.transpose(0, 2, 1, 3)
        # xmb0[(aL b), (nl j kq)] = mean_h x[nl, h, 4j+aL, kq] (exact f32)
        m0 = (xc.astype(np.float64).sum(axis=1) / HW).astype(np.float32)
        m0 = m0.reshape(NL, J, 4, PS).transpose(2, 0, 1, 3)  # aL,nl,j,kq
        xmb0 = np.broadcast_to(
            m0[:, None], (4, 32, NL, J, PS)).reshape(128, G * PS)
        # xs16t[g, kq, aL*256 + h] = x[nl, h, 4j+aL, kq];  g = nl*8 + j
        xj = xc.reshape(NL, HW, J, 4, PS)              # (nl,h,j,aL,kq)
        xs16t = xj.transpose(0, 2, 4, 3, 1).astype(np.float16)  # nl,j,kq,aL,h
        in_maps.append({
            "xs16t": np.ascontiguousarray(xs16t.reshape(G, PS, 4 * HW)),
            "xall": np.ascontiguousarray(xa.reshape(2 * NL, 128, 2 * A * PS)),
            "wga": np.ascontiguousarray(np.concatenate(
                [wga, xmb0.astype(np.float16)], axis=1)),
            "wws": wws,
        })
    return in_maps


_NC_CACHE = {}


def kernel(x, weights):
    if "nc" not in _NC_CACHE:
        _NC_CACHE["nc"] = _build_kernel()
    nc = _NC_CACHE["nc"]
    in_maps = _host_prep(x, weights)
    res = run_bass_kernel_spmd(nc, in_maps, list(range(NCORES)))
    out = np.concatenate([res.results[c]["o32"] for c in range(NCORES)],
                         axis=0)
    return out.astype(np.float32)

